# revision 6
# baseline (speedup 1.0000x reference)
# Binarized 3x3 conv (per-direction / population-parallel), Trainium2 Bass kernel.
#
# Reference math: bits {0,1} -> {-1,+1}; out = 4*xw - 2*sx - 2*sw + K.
# Identity used here:  out = conv(x, W4) - T2
#   where W4 = 4w - 2 (values +-2, exact in fp8e4), T2[cout] = sum (2w-1),
#   conv is a standard zero-padded 3x3 conv with x in {0,1}.
# Proof: sum(x*(4w-2)) - sum(2w-1) = 4xw - 2sx - (2sw - K).
# Output values are integers in [-1152, 1152] -> exact in fp16.
#
# Sharding: D=64 directions split 8 per core across 8 NeuronCores (pure
# population parallelism, no communication).
#
# All data conditioning happens on the host (not part of the HW kernel):
# x is uploaded as a zero-padded channel-major fp8 {0,1} image [34, 34],
# w as fp8 W4 = 4w-2 in [cin, tap, cout] with taps permuted so DoubleRow
# pairs are adjacent, and the -T2 bias as f32 [cout, DPC].
#
# The device runs the conv as fp8 DoubleRow matmuls: two taps per matmul
# (2 fp8 weights per PE cell, 2x throughput), 4 pairs + 1 normal tap per
# 512-pixel block, accumulating [cout, pix] in PSUM.  The rhs pair planes
# are raw 4D access patterns over the padded image (pair stride = tap
# offset delta).
#
# Engine/queue layout (v2): no ACTIVATE anywhere (avoids the ~1.3us
# ACT_TABLE_LOAD the backend would put at the head of scalar's queue);
# every epilogue is a DVE tensor_scalar into fp16, the final block splits
# its epilogue DVE/GpSimd and its output DMA across both HWDGE queues so
# the exposed tail after the last matmul is minimal.  Inputs: x on the
# sync HWDGE queue, w on scalar (early dirs) + gpsimd SWDGE (late dirs),
# block-0 outputs on scalar, block-1 outputs on sync.  Warmup matmuls on
# a memset scratch cover the ~1.5us DMA ring-startup window so the PE
# clock gate (HAM, 1.2 -> 2.4 GHz after ~3.4us sustained busy) unthrottles
# as early as possible.  The host transposes [cout, pix] fp16 back to
# [pix, cout] f32 (exact, integer values).

import numpy as np

import concourse.bass as bass
import concourse.mybir as mybir
import concourse.tile as tile
from concourse import bacc
from concourse import bass_utils

N_CORES = 8
D, H, W, CIN, COUT = 64, 32, 32, 128, 128
DPC = D // N_CORES  # directions per core
NPIX = H * W  # 1024
IMH, IMW = 34, 34  # padded image
IMSZ = IMH * IMW  # 1156
WSZ = 9 * COUT  # 1152

FP32 = mybir.dt.float32
FP16 = mybir.dt.float16
BF16 = mybir.dt.bfloat16
FP8 = mybir.dt.float8e4
I8 = mybir.dt.int8

ONE_FP8 = 0x38  # 1.0 in e4m3
POS2_FP8 = 0x40  # 2.0
NEG2_FP8 = 0xC0  # -2.0

# Tap order in the uploaded weight buffer: DoubleRow pairs adjacent.
# (i, j) = (filter row, filter col); window offset in image = i*34 + j.
TAP_PERM = [(0, 0), (0, 1), (1, 0), (1, 1), (2, 0), (2, 1), (0, 2), (1, 2), (2, 2)]
N_WARMUP = 8  # coarse N=256 warmups
N_TRAILER = 6  # fine N=64 trailer warmups for a tight handoff to real work


def _body(nc, tc, x_d, w_d, t_d, o_d):
    Alu = mybir.AluOpType
    DR = mybir.MatmulPerfMode.DoubleRow
    with (
        tc.tile_pool(name="const", bufs=1) as constp,
        tc.tile_pool(name="of", bufs=2 * DPC, space="SBUF") as ofp,
        tc.tile_pool(name="psA", bufs=4, space="PSUM") as psA,
        tc.tile_pool(name="psW", bufs=1, space="PSUM") as psW,
    ):
        # PE warmup: HAM un-throttles (1.2 -> 2.4 GHz) only after ~3.4us of
        # sustained matmul activity; burn the DMA-fill window on scratch
        # matmuls so the window starts counting as early as possible, with
        # small N=64 trailers at the end so the handoff to the first real
        # matmul (data-gated, ~9.2us) leaves no PE-idle gap that would
        # restart the HAM busy window.
        scratch = constp.tile([128, 256], BF16)
        nc.vector.memset(scratch, 0.0)
        wacc = psW.tile([128, 256], FP32)
        for _ in range(N_WARMUP):
            nc.tensor.matmul(
                wacc, lhsT=scratch[:, 0:128], rhs=scratch, start=True, stop=True
            )
        for _ in range(N_TRAILER):
            nc.tensor.matmul(
                wacc[:, 0:64], lhsT=scratch[:, 0:128], rhs=scratch[:, 0:64],
                start=True, stop=True,
            )

        # All input DMAs issued upfront (SBUF easily fits every direction).
        # Early-phase HWDGE throughput is only ~90 GB/s per queue with a
        # ~1.5us descgen+ring-startup latency, so direction 0's working
        # set is split into small chunks ordered exactly by first use:
        # w0 tap pairs on scalar, x0 window slices on sync.
        xall = constp.tile([128, DPC, IMSZ], I8)
        wall = constp.tile([128, DPC, 9, COUT], I8)
        negT = constp.tile([128, DPC], FP32)
        x0 = x_d[0].rearrange("c h w -> c (h w)")
        x0sb = xall[:, 0]
        w0 = w_d[0].rearrange("c t o -> c (t o)")
        w0sb = wall[:, 0].rearrange("p t o -> p (t o)")
        nc.scalar.dma_start(w0sb[:, 0:256], w0[:, 0:256])  # taps 0-1
        nc.sync.dma_start(x0sb[:, 0:340], x0[:, 0:340])  # rows 0-9
        nc.scalar.dma_start(w0sb[:, 256:512], w0[:, 256:512])  # taps 2-3
        nc.sync.dma_start(x0sb[:, 340:612], x0[:, 340:612])  # rows 10-17
        nc.scalar.dma_start(w0sb[:, 512:1152], w0[:, 512:1152])  # taps 4-8
        nc.sync.dma_start(x0sb[:, 612:1156], x0[:, 612:1156])  # rows 18-33
        nc.gpsimd.dma_start(negT, t_d)
        for d in range(1, DPC):
            nc.sync.dma_start(xall[:, d], x_d[d].rearrange("c h w -> c (h w)"))
        for d in range(1, 5):
            nc.scalar.dma_start(wall[:, d], w_d[d])
        for d in range(5, DPC):
            nc.gpsimd.dma_start(wall[:, d], w_d[d])

        def emit_conv(d, xim, pstride, ob, row0, col0, nrows):
            # 9-tap conv over pixel rows [row0, row0+nrows) into psum
            # columns [col0, col0+32*nrows): 4 DoubleRow pair-matmuls + 1
            # normal.  The rhs pair AP reads both taps' windows (2nd plane
            # at +delta).
            obr = ob[:, col0 : col0 + 32 * nrows]
            for k in range(4):
                (i0, j0), (i1, j1) = TAP_PERM[2 * k], TAP_PERM[2 * k + 1]
                off = (row0 + i0) * IMW + j0
                delta = (i1 - i0) * IMW + (j1 - j0)
                rhs = bass.AP(
                    xim.tensor,
                    xim.offset + off,
                    [pstride, [delta, 2], [IMW, nrows], [1, 32]],
                )
                nc.tensor.matmul(
                    obr,
                    lhsT=wall[:, d, 2 * k : 2 * k + 2, :].bitcast(FP8),
                    rhs=rhs,
                    start=(k == 0), stop=False, perf_mode=DR,
                )
            i8, j8 = TAP_PERM[8]
            off = (row0 + i8) * IMW + j8
            rhs = bass.AP(
                xim.tensor, xim.offset + off, [pstride, [IMW, nrows], [1, 32]]
            )
            nc.tensor.matmul(
                obr, lhsT=wall[:, d, 8, :].bitcast(FP8), rhs=rhs,
                start=False, stop=True
            )

        for d in range(DPC):
            xim = xall[:, d].bitcast(FP8)
            pstride = xim.ap[0]
            bias = negT[:, d : d + 1]
            od = o_d[d].rearrange("c (b n) -> c b n", b=2)
            for b in range(2):
                ob = psA.tile([128, 512], FP32, tag="acc", name=f"acc{d}{b}")
                if d == 0 and b == 0:
                    # First block runs as two half-blocks so the very first
                    # matmul only needs x rows 0-9 + w taps 0-1 (the first
                    # DMA chunks) — work starts ~0.7us earlier.
                    emit_conv(d, xim, pstride, ob, 0, 0, 8)
                    emit_conv(d, xim, pstride, ob, 8, 256, 8)
                else:
                    emit_conv(d, xim, pstride, ob, 16 * b, 0, 16)
                # Epilogue: out = acc - T2, fp16 (exact: integers <= 1152),
                # always on DVE (hidden under the next blocks' matmuls).
                last = d == DPC - 1 and b == 1
                if not last:
                    ofb = ofp.tile([128, 512], FP16, tag="of", name=f"of{d}{b}")
                    nc.vector.tensor_scalar(ofb, ob, 1.0, bias, Alu.mult, Alu.add)
                    if b == 0:
                        nc.scalar.dma_start(od[:, 0], ofb)
                    else:
                        nc.sync.dma_start(od[:, 1], ofb)
                else:
                    # Final block: nothing left to hide behind (GpSimd can't
                    # read PSUM), so run the epilogue as two DVE chunks and
                    # fan each chunk's output DMA to its own HWDGE queue as
                    # soon as it lands.
                    ofb = ofp.tile([128, 512], FP16, tag="of", name="oflast")
                    ol = od[:, 1]
                    nc.vector.tensor_scalar(
                        ofb[:, 0:256], ob[:, 0:256], 1.0, bias, Alu.mult, Alu.add
                    )
                    nc.scalar.dma_start(ol[:, 0:256], ofb[:, 0:256])
                    nc.vector.tensor_scalar(
                        ofb[:, 256:512], ob[:, 256:512], 1.0, bias, Alu.mult, Alu.add
                    )
                    nc.sync.dma_start(ol[:, 256:512], ofb[:, 256:512])


_NC_CACHE = None


def _get_nc():
    global _NC_CACHE
    if _NC_CACHE is None:
        nc = bacc.Bacc(
            "TRN2", target_bir_lowering=False, debug=False, num_devices=N_CORES
        )
        x_d = nc.dram_tensor(
            "x_s", [DPC, CIN, IMH, IMW], I8, kind="ExternalInput"
        ).ap()
        w_d = nc.dram_tensor(
            "w_s", [DPC, CIN, 9, COUT], I8, kind="ExternalInput"
        ).ap()
        t_d = nc.dram_tensor("t_s", [COUT, DPC], FP32, kind="ExternalInput").ap()
        o_d = nc.dram_tensor(
            "out_s", [DPC, COUT, NPIX], FP16, kind="ExternalOutput"
        ).ap()
        with tile.TileContext(nc) as tc:
            _body(nc, tc, x_d, w_d, t_d, o_d)
        nc.compile()
        _NC_CACHE = nc
    return _NC_CACHE


def _in_maps(x, w):
    # x: [D,H,W,CIN] bool -> zero-padded channel-major fp8 {0,1} image.
    xb = np.ascontiguousarray(x).view(np.uint8)  # 0/1
    xim = np.zeros((D, CIN, IMH, IMW), np.uint8)
    xim[:, :, 1 : H + 1, 1 : W + 1] = (
        np.transpose(xb, (0, 3, 1, 2)) * np.uint8(ONE_FP8)
    )
    xim = xim.view(np.int8)

    # w: [D,3,3,CIN,COUT] f32 {0,1} -> fp8 W4 = 4w-2 in [cin, tap, cout],
    # taps ordered per TAP_PERM (DoubleRow pairs adjacent).
    wb = np.ascontiguousarray(w) > 0.5
    w4 = np.where(wb, np.uint8(POS2_FP8), np.uint8(NEG2_FP8))
    perm = [3 * i + j for (i, j) in TAP_PERM]
    w4 = np.ascontiguousarray(
        np.transpose(w4.reshape(D, 9, CIN, COUT)[:, perm], (0, 2, 1, 3))
    ).view(np.int8)

    # -T2[cout] = -(2*sum(w) - K), pre-transposed to [cout, D].
    sw = wb.sum(axis=(1, 2, 3), dtype=np.int32)  # [D, COUT]
    negT = np.ascontiguousarray((9 * CIN - 2 * sw).astype(np.float32).T)

    return [
        {
            "x_s": xim[c * DPC : (c + 1) * DPC],
            "w_s": w4[c * DPC : (c + 1) * DPC],
            "t_s": negT[:, c * DPC : (c + 1) * DPC],
        }
        for c in range(N_CORES)
    ]


def kernel(x, w, _trace=False):
    nc = _get_nc()
    res = bass_utils.run_bass_kernel_spmd(
        nc, _in_maps(x, w), core_ids=list(range(N_CORES)), trace=_trace
    )
    out = np.concatenate([r["out_s"] for r in res.results], axis=0)
    # [D, COUT, NPIX] fp16 -> [D, H, W, COUT] f32 (exact: integer values)
    out = np.transpose(out, (0, 2, 1)).reshape(D, H, W, COUT).astype(np.float32)
    if _trace:
        return out, res
    return out


# revision 7
# speedup vs baseline: 1.1749x; 1.1749x over previous
# Binarized 3x3 conv (per-direction / population-parallel), Trainium2 Bass kernel.
#
# Reference math: bits {0,1} -> {-1,+1}; out = 4*xw - 2*sx - 2*sw + K.
# Identity used here:  out = conv(x, W4) - T2
#   where W4 = 4w - 2 (values +-2, exact in fp8e4), T2[cout] = sum (2w-1),
#   conv is a standard zero-padded 3x3 conv with x in {0,1}.
# Proof: sum(x*(4w-2)) - sum(2w-1) = 4xw - 2sx - (2sw - K).
# Output values are integers in [-1152, 1152] -> exact in fp16.
#
# Sharding: D=64 directions split 8 per core across 8 NeuronCores (pure
# population parallelism, no communication).
#
# Device pipeline:
#   - conv as fp8 DoubleRow matmuls: two taps per matmul (2 fp8 weights
#     per PE cell, 2x throughput), 4 pairs + 1 normal tap per 512-pixel
#     block, accumulating [cout, pix] in PSUM.  Rhs pair planes are raw
#     4D access patterns over the padded image (pair stride = tap delta).
#   - epilogue out = acc - T2 into fp16 on DVE only (no ACTIVATE -> no
#     ~1.3us ACT_TABLE_LOAD at the head of scalar's queue).
#   - HAM warmup: scratch matmuls (coarse + fine trailers) cover the DMA
#     fill window with no PE-idle gap, so the 1.2->2.4 GHz clock ungate
#     (needs ~3.4-4us sustained busy) fires as early as possible.
#
# Input staging (the critical path at startup): early-phase DMA rate is
# packet-bound (~12ns/packet, worse for small strided HBM reads), so all
# inputs are uploaded as CONTIGUOUS chunk tensors sized for 2KB+ packets
# and DMA'd whole: a head chunk with exactly direction 0's first working
# set (x rows 0-17, w taps 0-5), the rest of direction 0, then one
# [x_d | w_d] chunk per direction.  Queues: sync + scalar (HWDGE) carry
# the early chunks split pairwise; gpsimd (SWDGE) carries the bias and
# the last direction's chunk.  Outputs: block-0 on scalar, block-1 on
# sync; the last direction's final block is split into two half-blocks
# so only a 256-pixel epilogue + DMA is exposed after the last matmul.

import numpy as np

import concourse.bass as bass
import concourse.mybir as mybir
import concourse.tile as tile
from concourse import bacc
from concourse import bass_utils

N_CORES = 8
D, H, W, CIN, COUT = 64, 32, 32, 128, 128
DPC = D // N_CORES  # directions per core
NPIX = H * W  # 1024
IMH, IMW = 34, 34  # padded image
IMSZ = IMH * IMW  # 1156
WSZ = 9 * COUT  # 1152
DSZ = IMSZ + WSZ  # 2308 bytes per direction per partition
XH = 18 * IMW  # 612: x cols needed by block 0 (rows 0-17)
WH = 6 * COUT  # 768: w cols for tap pairs 0-2 (taps 0-5)

FP32 = mybir.dt.float32
FP16 = mybir.dt.float16
BF16 = mybir.dt.bfloat16
FP8 = mybir.dt.float8e4
I8 = mybir.dt.int8

ONE_FP8 = 0x38  # 1.0 in e4m3
POS2_FP8 = 0x40  # 2.0
NEG2_FP8 = 0xC0  # -2.0

# Tap order in the uploaded weight buffer: DoubleRow pairs adjacent.
# (i, j) = (filter row, filter col); window offset in image = i*34 + j.
TAP_PERM = [(0, 0), (0, 1), (1, 0), (1, 1), (2, 0), (2, 1), (0, 2), (1, 2), (2, 2)]
N_WARMUP = 5  # coarse N=256 warmups
N_TRAILER = 6  # fine N=64 trailer warmups for a tight handoff


def _body(nc, tc, xh_d, x0r_d, wh_d, w0r_d, dp_d, t_d, o_d):
    Alu = mybir.AluOpType
    DR = mybir.MatmulPerfMode.DoubleRow
    with (
        tc.tile_pool(name="const", bufs=1) as constp,
        tc.tile_pool(name="of", bufs=2 * DPC, space="SBUF") as ofp,
        tc.tile_pool(name="psA", bufs=4, space="PSUM") as psA,
        tc.tile_pool(name="psW", bufs=1, space="PSUM") as psW,
    ):
        scratch = constp.tile([128, 256], BF16)
        nc.vector.memset(scratch, 0.0)
        wacc = psW.tile([128, 256], FP32)
        for _ in range(N_WARMUP):
            nc.tensor.matmul(
                wacc, lhsT=scratch[:, 0:128], rhs=scratch, start=True, stop=True
            )
        for _ in range(N_TRAILER):
            nc.tensor.matmul(
                wacc[:, 0:64], lhsT=scratch[:, 0:128], rhs=scratch[:, 0:64],
                start=True, stop=True,
            )

        # One flat input tile; per direction: [x image 1156 | w taps 1152].
        allin = constp.tile([128, DPC * DSZ], I8)
        negT = constp.tile([128, DPC], FP32)
        nc.sync.dma_start(allin[:, 0:XH], xh_d)
        nc.scalar.dma_start(allin[:, IMSZ : IMSZ + WH], wh_d)
        nc.sync.dma_start(allin[:, XH:IMSZ], x0r_d)
        nc.scalar.dma_start(allin[:, IMSZ + WH : DSZ], w0r_d)
        nc.gpsimd.dma_start(negT, t_d)
        for d in range(1, DPC):
            dst = allin[:, d * DSZ : (d + 1) * DSZ]
            if d == DPC - 1:
                nc.gpsimd.dma_start(dst, dp_d[d - 1])
            elif d % 2 == 1:
                nc.scalar.dma_start(dst, dp_d[d - 1])
            else:
                nc.sync.dma_start(dst, dp_d[d - 1])

        def emit_conv(wv, xim, pstride, ob, row0, col0, nrows):
            # 9-tap conv over pixel rows [row0, row0+nrows) into psum
            # columns [col0, col0+32*nrows): 4 DoubleRow pair-matmuls + 1
            # normal.  The rhs pair AP reads both taps' windows (2nd plane
            # at +delta).
            obr = ob[:, col0 : col0 + 32 * nrows]
            for k in range(4):
                (i0, j0), (i1, j1) = TAP_PERM[2 * k], TAP_PERM[2 * k + 1]
                off = (row0 + i0) * IMW + j0
                delta = (i1 - i0) * IMW + (j1 - j0)
                rhs = bass.AP(
                    xim.tensor,
                    xim.offset + off,
                    [pstride, [delta, 2], [IMW, nrows], [1, 32]],
                )
                nc.tensor.matmul(
                    obr,
                    lhsT=wv[:, 2 * k : 2 * k + 2, :].bitcast(FP8),
                    rhs=rhs,
                    start=(k == 0), stop=False, perf_mode=DR,
                )
            i8, j8 = TAP_PERM[8]
            off = (row0 + i8) * IMW + j8
            rhs = bass.AP(
                xim.tensor, xim.offset + off, [pstride, [IMW, nrows], [1, 32]]
            )
            nc.tensor.matmul(
                obr, lhsT=wv[:, 8, :].bitcast(FP8), rhs=rhs,
                start=False, stop=True
            )

        for d in range(DPC):
            xim = allin[:, d * DSZ : d * DSZ + IMSZ].bitcast(FP8)
            wv = allin[:, d * DSZ + IMSZ : (d + 1) * DSZ].rearrange(
                "p (t o) -> p t o", t=9
            )
            pstride = xim.ap[0]
            bias = negT[:, d : d + 1]
            od = o_d[d].rearrange("c (b n) -> c b n", b=2)
            for b in range(2):
                ob = psA.tile([128, 512], FP32, tag="acc", name=f"acc{d}{b}")
                last = d == DPC - 1 and b == 1
                if not last:
                    emit_conv(wv, xim, pstride, ob, 16 * b, 0, 16)
                    # Epilogue: out = acc - T2, fp16 (exact: integers
                    # <= 1152), on DVE, hidden under the next block's MMs.
                    ofb = ofp.tile([128, 512], FP16, tag="of", name=f"of{d}{b}")
                    nc.vector.tensor_scalar(ofb, ob, 1.0, bias, Alu.mult, Alu.add)
                    if b == 0:
                        nc.scalar.dma_start(od[:, 0], ofb)
                    else:
                        nc.sync.dma_start(od[:, 1], ofb)
                else:
                    # Final block: two half-blocks so half the epilogue and
                    # output DMA hide under the second half's matmuls; only
                    # a 256-pixel epilogue + fanned-out DMA stays exposed.
                    ofb = ofp.tile([128, 512], FP16, tag="of", name="oflast")
                    ol = od[:, 1]
                    emit_conv(wv, xim, pstride, ob, 16, 0, 8)
                    nc.vector.tensor_scalar(
                        ofb[:, 0:256], ob[:, 0:256], 1.0, bias, Alu.mult, Alu.add
                    )
                    nc.sync.dma_start(ol[:, 0:256], ofb[:, 0:256])
                    emit_conv(wv, xim, pstride, ob, 24, 256, 8)
                    nc.vector.tensor_scalar(
                        ofb[:, 256:512], ob[:, 256:512], 1.0, bias, Alu.mult, Alu.add
                    )
                    nc.scalar.dma_start(ol[:, 256:384], ofb[:, 256:384])
                    nc.sync.dma_start(ol[:, 384:512], ofb[:, 384:512])


_NC_CACHE = None


def _get_nc():
    global _NC_CACHE
    if _NC_CACHE is None:
        nc = bacc.Bacc(
            "TRN2", target_bir_lowering=False, debug=False, num_devices=N_CORES
        )
        xh_d = nc.dram_tensor("xh_s", [CIN, XH], I8, kind="ExternalInput").ap()
        x0r_d = nc.dram_tensor(
            "x0r_s", [CIN, IMSZ - XH], I8, kind="ExternalInput"
        ).ap()
        wh_d = nc.dram_tensor("wh_s", [CIN, WH], I8, kind="ExternalInput").ap()
        w0r_d = nc.dram_tensor(
            "w0r_s", [CIN, WSZ - WH], I8, kind="ExternalInput"
        ).ap()
        dp_d = nc.dram_tensor(
            "dp_s", [DPC - 1, CIN, DSZ], I8, kind="ExternalInput"
        ).ap()
        t_d = nc.dram_tensor("t_s", [COUT, DPC], FP32, kind="ExternalInput").ap()
        o_d = nc.dram_tensor(
            "out_s", [DPC, COUT, NPIX], FP16, kind="ExternalOutput"
        ).ap()
        with tile.TileContext(nc) as tc:
            _body(nc, tc, xh_d, x0r_d, wh_d, w0r_d, dp_d, t_d, o_d)
        nc.compile()
        _NC_CACHE = nc
    return _NC_CACHE


def _in_maps(x, w):
    # x: [D,H,W,CIN] bool -> zero-padded channel-major fp8 {0,1} image.
    xb = np.ascontiguousarray(x).view(np.uint8)  # 0/1
    xim = np.zeros((D, CIN, IMH * IMW), np.uint8)
    xim.reshape(D, CIN, IMH, IMW)[:, :, 1 : H + 1, 1 : W + 1] = (
        np.transpose(xb, (0, 3, 1, 2)) * np.uint8(ONE_FP8)
    )

    # w: [D,3,3,CIN,COUT] f32 {0,1} -> fp8 W4 = 4w-2 in [cin, tap*cout],
    # taps ordered per TAP_PERM (DoubleRow pairs adjacent).
    wb = np.ascontiguousarray(w) > 0.5
    w4 = np.where(wb, np.uint8(POS2_FP8), np.uint8(NEG2_FP8))
    perm = [3 * i + j for (i, j) in TAP_PERM]
    w4 = np.ascontiguousarray(
        np.transpose(w4.reshape(D, 9, CIN, COUT)[:, perm], (0, 2, 1, 3))
    ).reshape(D, CIN, WSZ)

    # Per-direction contiguous [x | w] chunks (2308-byte DMA packets).
    dp = np.concatenate([xim, w4], axis=2)  # [D, CIN, DSZ] uint8

    # -T2[cout] = -(2*sum(w) - K), pre-transposed to [cout, D].
    sw = wb.sum(axis=(1, 2, 3), dtype=np.int32)  # [D, COUT]
    negT = np.ascontiguousarray((9 * CIN - 2 * sw).astype(np.float32).T)

    maps = []
    for c in range(N_CORES):
        d0 = c * DPC
        maps.append(
            {
                "xh_s": np.ascontiguousarray(xim[d0, :, 0:XH]).view(np.int8),
                "x0r_s": np.ascontiguousarray(xim[d0, :, XH:IMSZ]).view(np.int8),
                "wh_s": np.ascontiguousarray(w4[d0, :, 0:WH]).view(np.int8),
                "w0r_s": np.ascontiguousarray(w4[d0, :, WH:WSZ]).view(np.int8),
                "dp_s": np.ascontiguousarray(dp[d0 + 1 : d0 + DPC]).view(np.int8),
                "t_s": negT[:, d0 : d0 + DPC],
            }
        )
    return maps


def kernel(x, w, _trace=False):
    nc = _get_nc()
    res = bass_utils.run_bass_kernel_spmd(
        nc, _in_maps(x, w), core_ids=list(range(N_CORES)), trace=_trace
    )
    out = np.concatenate([r["out_s"] for r in res.results], axis=0)
    # [D, COUT, NPIX] fp16 -> [D, H, W, COUT] f32 (exact: integer values)
    out = np.transpose(out, (0, 2, 1)).reshape(D, H, W, COUT).astype(np.float32)
    if _trace:
        return out, res
    return out


# revision 11
# speedup vs baseline: 1.2479x; 1.0621x over previous
# Binarized 3x3 conv (per-direction / population-parallel), Trainium2 Bass kernel.
#
# Reference math: bits {0,1} -> {-1,+1}; out = 4*xw - 2*sx - 2*sw + K.
# Identity used here:  out = conv(x, W4) - T2
#   where W4 = 4w - 2 (values +-2, exact in fp8e4), T2[cout] = sum (2w-1),
#   conv is a standard zero-padded 3x3 conv with x in {0,1}.
# Proof: sum(x*(4w-2)) - sum(2w-1) = 4xw - 2sx - (2sw - K).
# Output values are integers in [-1152, 1152] -> exact in fp16.
#
# Sharding: D=64 directions split 8 per core across 8 NeuronCores (pure
# population parallelism, no communication).
#
# Device pipeline:
#   - conv as fp8 DoubleRow matmuls: two taps per matmul (2 fp8 weights
#     per PE cell, 2x throughput), 4 pairs + 1 normal tap per 512-pixel
#     block, accumulating [cout, pix] in PSUM.  Rhs pair planes are raw
#     4D access patterns over the padded image (pair stride = tap delta).
#   - epilogue out = acc - T2 into fp16 on DVE only (no ACTIVATE -> no
#     ~1.3us ACT_TABLE_LOAD at the head of scalar's queue).
#   - HAM warmup: scratch matmuls (coarse + fine trailers) cover the DMA
#     fill window with no PE-idle gap, so the 1.2->2.4 GHz clock ungate
#     (needs ~3.4-4us sustained busy) fires as early as possible.
#
# Input staging (the critical path at startup): early-phase DMA rate is
# packet-bound (~12ns/packet, worse for small strided HBM reads), so all
# inputs are uploaded as CONTIGUOUS chunk tensors sized for 2KB+ packets
# and DMA'd whole: a head chunk with exactly direction 0's first working
# set (x rows 0-17, w taps 0-5), the rest of direction 0, then one
# [x_d | w_d] chunk per direction.  Queues: sync + scalar (HWDGE) carry
# the early chunks split pairwise; gpsimd (SWDGE) carries the bias and
# the last direction's chunk.  Outputs: block-0 on scalar, block-1 on
# sync; the last direction's final block is split into two half-blocks
# so only a 256-pixel epilogue + DMA is exposed after the last matmul.

import numpy as np

import concourse.bass as bass
import concourse.mybir as mybir
import concourse.tile as tile
from concourse import bacc
from concourse import bass_utils

N_CORES = 8
D, H, W, CIN, COUT = 64, 32, 32, 128, 128
DPC = D // N_CORES  # directions per core
NPIX = H * W  # 1024
IMH, IMW = 34, 34  # padded image
IMSZ = IMH * IMW  # 1156
WSZ = 9 * COUT  # 1152
DSZ = IMSZ + WSZ  # 2308 bytes per direction per partition
XH = 18 * IMW  # 612: x cols needed by block 0 (rows 0-17)
WH = 6 * COUT  # 768: w cols for tap pairs 0-2 (taps 0-5)

FP32 = mybir.dt.float32
FP16 = mybir.dt.float16
BF16 = mybir.dt.bfloat16
FP8 = mybir.dt.float8e4
I8 = mybir.dt.int8

ONE_FP8 = 0x38  # 1.0 in e4m3
POS2_FP8 = 0x40  # 2.0
NEG2_FP8 = 0xC0  # -2.0

# Tap order in the uploaded weight buffer: DoubleRow pairs adjacent.
# (i, j) = (filter row, filter col); window offset in image = i*34 + j.
TAP_PERM = [(0, 0), (0, 1), (1, 0), (1, 1), (2, 0), (2, 1), (0, 2), (1, 2), (2, 2)]
N_WARMUP = 7  # coarse N=256 warmups
N_TRAILER = 8  # fine N=128 trailer warmups for a tight handoff


def _body(nc, tc, xh_d, x0r_d, wh_d, w0r_d, dp_d, t_d, o_d):
    Alu = mybir.AluOpType
    DR = mybir.MatmulPerfMode.DoubleRow
    with (
        tc.tile_pool(name="const", bufs=1) as constp,
        tc.tile_pool(name="of", bufs=2 * DPC, space="SBUF") as ofp,
        tc.tile_pool(name="psA", bufs=4, space="PSUM") as psA,
        tc.tile_pool(name="psW", bufs=1, space="PSUM") as psW,
    ):
        scratch = constp.tile([128, 256], BF16)
        nc.vector.memset(scratch, 0.0)
        wacc = psW.tile([128, 256], FP32)
        for _ in range(N_WARMUP):
            nc.tensor.matmul(
                wacc, lhsT=scratch[:, 0:128], rhs=scratch, start=True, stop=True
            )
        for _ in range(N_TRAILER):
            nc.tensor.matmul(
                wacc[:, 0:128], lhsT=scratch[:, 0:128], rhs=scratch[:, 0:128],
                start=True, stop=True,
            )

        # One flat input tile; per direction: [x image 1156 | w taps 1152].
        allin = constp.tile([128, DPC * DSZ], I8)
        negT = constp.tile([128, DPC], FP32)
        nc.sync.dma_start(allin[:, 0:XH], xh_d)
        nc.scalar.dma_start(allin[:, IMSZ : IMSZ + WH], wh_d)
        nc.sync.dma_start(allin[:, XH:IMSZ], x0r_d)
        nc.scalar.dma_start(allin[:, IMSZ + WH : DSZ], w0r_d)
        nc.gpsimd.dma_start(negT, t_d)
        for d in range(1, DPC):
            dst = allin[:, d * DSZ : (d + 1) * DSZ]
            if d == DPC - 1:
                nc.gpsimd.dma_start(dst, dp_d[d - 1])
            elif d % 2 == 1:
                nc.sync.dma_start(dst, dp_d[d - 1])
            else:
                nc.scalar.dma_start(dst, dp_d[d - 1])

        def emit_conv(wv, xim, pstride, ob, row0, col0, nrows):
            # 9-tap conv over pixel rows [row0, row0+nrows) into psum
            # columns [col0, col0+32*nrows): 4 DoubleRow pair-matmuls + 1
            # normal.  The rhs pair AP reads both taps' windows (2nd plane
            # at +delta).
            obr = ob[:, col0 : col0 + 32 * nrows]
            for k in range(4):
                (i0, j0), (i1, j1) = TAP_PERM[2 * k], TAP_PERM[2 * k + 1]
                off = (row0 + i0) * IMW + j0
                delta = (i1 - i0) * IMW + (j1 - j0)
                rhs = bass.AP(
                    xim.tensor,
                    xim.offset + off,
                    [pstride, [delta, 2], [IMW, nrows], [1, 32]],
                )
                nc.tensor.matmul(
                    obr,
                    lhsT=wv[:, 2 * k : 2 * k + 2, :].bitcast(FP8),
                    rhs=rhs,
                    start=(k == 0), stop=False, perf_mode=DR,
                )
            i8, j8 = TAP_PERM[8]
            off = (row0 + i8) * IMW + j8
            rhs = bass.AP(
                xim.tensor, xim.offset + off, [pstride, [IMW, nrows], [1, 32]]
            )
            nc.tensor.matmul(
                obr, lhsT=wv[:, 8, :].bitcast(FP8), rhs=rhs,
                start=False, stop=True
            )

        for d in range(DPC):
            xim = allin[:, d * DSZ : d * DSZ + IMSZ].bitcast(FP8)
            wv = allin[:, d * DSZ + IMSZ : (d + 1) * DSZ].rearrange(
                "p (t o) -> p t o", t=9
            )
            pstride = xim.ap[0]
            bias = negT[:, d : d + 1]
            od = o_d[d].rearrange("c (b n) -> c b n", b=2)
            for b in range(2):
                ob = psA.tile([128, 512], FP32, tag="acc", name=f"acc{d}{b}")
                last = d == DPC - 1 and b == 1
                if not last:
                    emit_conv(wv, xim, pstride, ob, 16 * b, 0, 16)
                    # Epilogue: out = acc - T2, fp16 (exact: integers
                    # <= 1152), on DVE, hidden under the next block's MMs.
                    ofb = ofp.tile([128, 512], FP16, tag="of", name=f"of{d}{b}")
                    nc.vector.tensor_scalar(ofb, ob, 1.0, bias, Alu.mult, Alu.add)
                    if b == 0:
                        nc.scalar.dma_start(od[:, 0], ofb)
                    else:
                        nc.sync.dma_start(od[:, 1], ofb)
                else:
                    # Final block: two half-blocks in SEPARATE psum tiles
                    # (sharing one tile would serialize half B's matmuls
                    # behind half A's epilogue read), so half A's epilogue
                    # and output DMA hide under half B's matmuls; only a
                    # 256-pixel epilogue + one DMA stays exposed.
                    ofb = ofp.tile([128, 512], FP16, tag="of", name="oflast")
                    ol = od[:, 1]
                    obB = psA.tile([128, 256], FP32, tag="acc", name="accLB")
                    emit_conv(wv, xim, pstride, ob, 16, 0, 8)
                    nc.vector.tensor_scalar(
                        ofb[:, 0:256], ob[:, 0:256], 1.0, bias, Alu.mult, Alu.add
                    )
                    nc.sync.dma_start(ol[:, 0:256], ofb[:, 0:256])
                    emit_conv(wv, xim, pstride, obB, 24, 0, 8)
                    nc.vector.tensor_scalar(
                        ofb[:, 256:512], obB, 1.0, bias, Alu.mult, Alu.add
                    )
                    nc.scalar.dma_start(ol[:, 256:512], ofb[:, 256:512])


_NC_CACHE = None


def _get_nc():
    global _NC_CACHE
    if _NC_CACHE is None:
        nc = bacc.Bacc(
            "TRN2", target_bir_lowering=False, debug=False, num_devices=N_CORES
        )
        xh_d = nc.dram_tensor("xh_s", [CIN, XH], I8, kind="ExternalInput").ap()
        x0r_d = nc.dram_tensor(
            "x0r_s", [CIN, IMSZ - XH], I8, kind="ExternalInput"
        ).ap()
        wh_d = nc.dram_tensor("wh_s", [CIN, WH], I8, kind="ExternalInput").ap()
        w0r_d = nc.dram_tensor(
            "w0r_s", [CIN, WSZ - WH], I8, kind="ExternalInput"
        ).ap()
        dp_d = nc.dram_tensor(
            "dp_s", [DPC - 1, CIN, DSZ], I8, kind="ExternalInput"
        ).ap()
        t_d = nc.dram_tensor("t_s", [COUT, DPC], FP32, kind="ExternalInput").ap()
        o_d = nc.dram_tensor(
            "out_s", [DPC, COUT, NPIX], FP16, kind="ExternalOutput"
        ).ap()
        with tile.TileContext(nc) as tc:
            _body(nc, tc, xh_d, x0r_d, wh_d, w0r_d, dp_d, t_d, o_d)
        nc.compile()
        _NC_CACHE = nc
    return _NC_CACHE


def _in_maps(x, w):
    # x: [D,H,W,CIN] bool -> zero-padded channel-major fp8 {0,1} image.
    xb = np.ascontiguousarray(x).view(np.uint8)  # 0/1
    xim = np.zeros((D, CIN, IMH * IMW), np.uint8)
    xim.reshape(D, CIN, IMH, IMW)[:, :, 1 : H + 1, 1 : W + 1] = (
        np.transpose(xb, (0, 3, 1, 2)) * np.uint8(ONE_FP8)
    )

    # w: [D,3,3,CIN,COUT] f32 {0,1} -> fp8 W4 = 4w-2 in [cin, tap*cout],
    # taps ordered per TAP_PERM (DoubleRow pairs adjacent).
    wb = np.ascontiguousarray(w) > 0.5
    w4 = np.where(wb, np.uint8(POS2_FP8), np.uint8(NEG2_FP8))
    perm = [3 * i + j for (i, j) in TAP_PERM]
    w4 = np.ascontiguousarray(
        np.transpose(w4.reshape(D, 9, CIN, COUT)[:, perm], (0, 2, 1, 3))
    ).reshape(D, CIN, WSZ)

    # Per-direction contiguous [x | w] chunks (2308-byte DMA packets).
    dp = np.concatenate([xim, w4], axis=2)  # [D, CIN, DSZ] uint8

    # -T2[cout] = -(2*sum(w) - K), pre-transposed to [cout, D].
    sw = wb.sum(axis=(1, 2, 3), dtype=np.int32)  # [D, COUT]
    negT = np.ascontiguousarray((9 * CIN - 2 * sw).astype(np.float32).T)

    maps = []
    for c in range(N_CORES):
        d0 = c * DPC
        maps.append(
            {
                "xh_s": np.ascontiguousarray(xim[d0, :, 0:XH]).view(np.int8),
                "x0r_s": np.ascontiguousarray(xim[d0, :, XH:IMSZ]).view(np.int8),
                "wh_s": np.ascontiguousarray(w4[d0, :, 0:WH]).view(np.int8),
                "w0r_s": np.ascontiguousarray(w4[d0, :, WH:WSZ]).view(np.int8),
                "dp_s": np.ascontiguousarray(dp[d0 + 1 : d0 + DPC]).view(np.int8),
                "t_s": negT[:, d0 : d0 + DPC],
            }
        )
    return maps


def kernel(x, w, _trace=False):
    nc = _get_nc()
    res = bass_utils.run_bass_kernel_spmd(
        nc, _in_maps(x, w), core_ids=list(range(N_CORES)), trace=_trace
    )
    out = np.concatenate([r["out_s"] for r in res.results], axis=0)
    # [D, COUT, NPIX] fp16 -> [D, H, W, COUT] f32 (exact: integer values)
    out = np.transpose(out, (0, 2, 1)).reshape(D, H, W, COUT).astype(np.float32)
    if _trace:
        return out, res
    return out
